# revision 1
# baseline (speedup 1.0000x reference)
"""AxialAttention TRN2 kernel, v3.

Sharding: 8 cores = 4 batches x 2 head-groups (4 heads each), all-bf16 compute.

Per core, two software-pipelined blocks:

ROW block (16 chunks of 8 rows, p1 fused with row attention, 2-chunk lag):
  - qkv projection s-major (x-stationary matmuls, all biases folded in as a
    K=1 ones-matmul): psum [px, ch] drains to an SBUF s-major chunk tile.
  - the chunk tile is written to DRAM twice, both as contiguous rows:
    qkR [S, 512] row-order (q,k, feeds row-block transposes) and
    qkvC [S', 768] col-order (feeds the whole col block).
  - q,k channel-major tiles for scores come from XBAR transpose DMAs
    (qkR rows -> [128, 1024]), 4 per chunk, on the SP queue which carries
    ONLY transposes (XBAR transpose<->copy interleave corrupts data).
  - row attention per chunk: scores (contract d=64 on partitions) -> exp
    (Act) -> pair-stacked Z (partition-offset ones-matmuls, head0 rows 0:64
    / head1 64:128) -> one f32 reciprocal per pair (DVE) -> PV with
    UNNORMALIZED P (v taken from the s-major SBUF tile directly) -> drain =
    tensor_tensor MULT(psO, rz) into O_sb (DVE), so normalization is free.
  - O_sb is channel-major in col-major pixel order; row drains scatter.

COL block (16 chunks of 8 cols + output projection, pipelined):
  - q,k via transposes of qkvC rows; v via a plain strided load (512B rows).
  - drain: psO*rz on DVE into a temp, then SBUF-only add into O_sb on Pool
    (GPSIMD cannot touch PSUM).
  - p3 output projection per finished col chunk, bf16 out in col-major
    pixel order; bout and the final (W,H)->(H,W) transpose happen on host.

Engine/queue notes: DMAs occupy the issuing queue; only SP and Act have
HWDGE, Pool is SWDGE (<16K descriptors, casts allowed). All PSUM-reading
drains sit on Act/DVE. PE does only matmuls.
"""

import numpy as np
import ml_dtypes
from contextlib import ExitStack

import concourse.bass as bass
import concourse.bacc as bacc
import concourse.tile as tile
from concourse import mybir
from concourse.bass_utils import run_bass_kernel_spmd

C = 512          # channels
H = 128
W = 128
S = H * W        # 16384 pixels
NHC = 4          # heads per core
D = 64           # head dim
GC = NHC * D     # 256 q (or k, or v) channels per core
QK = 2 * GC      # 512 q+k channels
QKV = 3 * GC     # 768 qkv channels
CT = C // 128    # 4 contraction tiles
NCH = 16         # chunks (8 rows or 8 cols each)
SCALE = 1.0 / np.sqrt(D)

F32 = mybir.dt.float32
BF16 = mybir.dt.bfloat16
EXP = mybir.ActivationFunctionType.Exp
ADD = mybir.AluOpType.add
MULT = mybir.AluOpType.mult

_CACHED_NC = None


def _interleave(primary, secondary):
    """Emit primary unit list with secondary units spread evenly between."""
    np_, ns = len(primary), len(secondary)
    if np_ == 0:
        for u in secondary:
            u()
        return
    emitted = 0
    for i, u in enumerate(primary):
        u()
        want = (i + 1) * ns // np_
        while emitted < want:
            secondary[emitted]()
            emitted += 1
    while emitted < ns:
        secondary[emitted]()
        emitted += 1


def build_nc(reps=1):
    nc = bacc.Bacc()
    x_in = nc.dram_tensor("x", [C, S], BF16, kind="ExternalInput")
    wqkvT = nc.dram_tensor("wqkvT", [C, QKV], BF16, kind="ExternalInput")
    bqkv = nc.dram_tensor("bqkv", [1, QKV], BF16, kind="ExternalInput")
    woutT = nc.dram_tensor("woutT", [GC, C], BF16, kind="ExternalInput")
    out = nc.dram_tensor("out", [C, S], BF16, kind="ExternalOutput")

    with tile.TileContext(nc) as tc, ExitStack() as ctx:
        persist = ctx.enter_context(tc.tile_pool(name="persist", bufs=1))
        dram = ctx.enter_context(tc.tile_pool(name="dram", bufs=1, space="DRAM"))

        wqkv_sb = persist.tile([128, CT, QKV], BF16, tag="wqkv")
        nc.scalar.dma_start(
            out=wqkv_sb, in_=wqkvT.ap().rearrange("(t p) o -> p t o", p=128)
        )
        bqkv_sb = persist.tile([1, QKV], BF16, tag="bqkv")
        nc.scalar.dma_start(out=bqkv_sb, in_=bqkv.ap())
        wout_sb = persist.tile([128, 2, C], BF16, tag="wout")
        nc.scalar.dma_start(
            out=wout_sb, in_=woutT.ap().rearrange("(t p) o -> p t o", p=128)
        )
        ones1 = persist.tile([1, 128], BF16, tag="ones1")
        nc.vector.memset(ones1, 1.0)
        onesZ = persist.tile([128, 64], BF16, tag="onesZ")
        nc.vector.memset(onesZ, 1.0)
        O_sb = [
            persist.tile([128, S], BF16, tag=f"O{i}", name=f"O{i}")
            for i in range(2)
        ]

        qkR = dram.tile([S, QK], BF16, tag="qkR")     # q,k s-major row-order
        qkvC = dram.tile([S, QKV], BF16, tag="qkvC")  # qkv s-major col-order

        for _ in range(reps):
            build_body(nc, tc, x_in, wqkv_sb, bqkv_sb, wout_sb,
                       ones1, onesZ, O_sb, qkR, qkvC, out)

    nc.finalize()
    return nc


def build_body(nc, tc, x_in, wqkv_sb, bqkv_sb, wout_sb, ones1, onesZ,
               O_sb, qkR, qkvC, out):
    x_r = x_in.ap().rearrange("(t p) s -> p t s", p=128)
    qkR_w = qkR[:].rearrange("(t p) d -> p t d", p=128)
    # col-order row-chunk write view: partitions = w, then (h, ch)
    qkvC_w = qkvC[:].rearrange("(w h) d -> w h d", h=H)

    def make_attn_units(qk_t, v_t, voff, pchpool, rzpool, psSp, psZp, psOp,
                        drain_pair, s_halves):
        """One chunk of axial attention (4 heads as 2 pairs).
        qk_t: [128, 4, 1024] ch-major (cht 0,1 = q; 2,3 = k).
        v_t: [128, 8, *] s-major, head h at cols voff+h*64.
        s_halves: emit scores/exp at [*,512] half-granularity (col block's
        psS/psZ pools are [128,512] bufs=2; row's are [128,1024] bufs=1)."""
        pchs = {}
        rzs = {}
        units = []

        def s_unit(h):
            def emit():
                r0 = (h % 2) * 64
                cq = h // 2
                pch = pchpool.tile([128, 1024], BF16, name="pch")
                pchs[h] = pch
                nh = 2 if s_halves else 1
                for j in range(nh):
                    w_ = 1024 // nh
                    psS = psSp.tile([128, w_], F32, name="psS")
                    for ii in range(w_ // 128):
                        i = (w_ // 128) * j + ii
                        nc.tensor.matmul(
                            out=psS[:, ii * 128 : (ii + 1) * 128],
                            lhsT=qk_t[r0 : r0 + 64, 2 + cq, i * 128 : (i + 1) * 128],
                            rhs=qk_t[r0 : r0 + 64, cq, i * 128 : (i + 1) * 128],
                            start=True, stop=True,
                        )
                    nc.scalar.activation(
                        out=pch[:, j * w_ : (j + 1) * w_], in_=psS,
                        func=EXP, scale=float(SCALE),
                    )
            return emit

        def z_unit(p):
            def emit():
                rz = rzpool.tile([128, 1024], F32, name="rz")
                rzs[p] = rz
                nh = 2 if s_halves else 1
                for j in range(nh):
                    w_ = 1024 // nh
                    psZ = psZp.tile([128, w_], F32, name="psZ")
                    for hl in range(2):
                        r0 = hl * 64
                        for jj in range(w_ // 512):
                            c0 = j * w_ + jj * 512
                            nc.tensor.matmul(
                                out=psZ[r0 : r0 + 64, jj * 512 : jj * 512 + 512],
                                lhsT=onesZ,
                                rhs=pchs[2 * p + hl][:, c0 : c0 + 512],
                                start=True, stop=True,
                            )
                    nc.vector.reciprocal_approx_fast(
                        out=rz[:, j * w_ : (j + 1) * w_], in_=psZ
                    )
            return emit

        def pv_unit(p):
            def emit():
                psO_t = psOp.tile([128, 1024], F32, name="psO")
                for hl in range(2):
                    h = 2 * p + hl
                    r0 = hl * 64
                    for i in range(8):
                        nc.tensor.matmul(
                            out=psO_t[r0 : r0 + 64, i * 128 : (i + 1) * 128],
                            lhsT=v_t[:, i, voff + h * 64 : voff + (h + 1) * 64],
                            rhs=pchs[h][:, i * 128 : (i + 1) * 128],
                            start=True, stop=True,
                        )
                drain_pair(p, psO_t, rzs[p])
            return emit

        units.append(s_unit(0))
        units.append(s_unit(1))
        units.append(z_unit(0))
        units.append(s_unit(2))
        units.append(pv_unit(0))
        units.append(s_unit(3))
        units.append(z_unit(1))
        units.append(pv_unit(1))
        return units

    # ---------------- row block: p1 + row attention, lag-2 pipeline ----------
    with (
        tc.tile_pool(name="r_x", bufs=3) as xpool,
        tc.tile_pool(name="r_qkv", bufs=3) as qkvpool,
        tc.tile_pool(name="r_qkt", bufs=2) as qktpool,
        tc.tile_pool(name="r_pch", bufs=3) as pchpool,
        tc.tile_pool(name="r_rz", bufs=2) as rzpool,
        tc.tile_pool(name="r_psqk", bufs=1, space="PSUM") as psQKp,
        tc.tile_pool(name="r_psv", bufs=1, space="PSUM") as psVp,
        tc.tile_pool(name="r_pss", bufs=1, space="PSUM") as psSp,
        tc.tile_pool(name="r_psz", bufs=1, space="PSUM") as psZp,
        tc.tile_pool(name="r_pso", bufs=1, space="PSUM") as psOp,
    ):
        xgs = {}
        xgs[0] = xpool.tile([128, CT, 1024], BF16, name="xg")
        nc.gpsimd.dma_start(out=xgs[0], in_=x_r[:, :, 0:1024])
        qkvs = {}   # n -> s-major chunk tile
        qkts = {}   # n -> transposed ch-major q,k tile

        def make_p1_units(n):
            qkv = qkvpool.tile([128, 8, QKV], BF16, name="qkv")
            qkvs[n] = qkv
            units = []

            def prefetch():
                if n + 1 < NCH:
                    xg = xpool.tile([128, CT, 1024], BF16, name="xg")
                    xgs[n + 1] = xg
                    nc.gpsimd.dma_start(
                        out=xg, in_=x_r[:, :, (n + 1) * 1024 : (n + 2) * 1024]
                    )
            units.append(prefetch)

            def qk_unit(pt):
                def emit():
                    ps = psQKp.tile([128, QK], F32)
                    for ct in range(CT):
                        nc.tensor.matmul(
                            out=ps,
                            lhsT=xgs[n][:, ct, pt * 128 : (pt + 1) * 128],
                            rhs=wqkv_sb[:, ct, 0:QK],
                            start=(ct == 0), stop=False,
                        )
                    nc.tensor.matmul(
                        out=ps, lhsT=ones1, rhs=bqkv_sb[:, 0:QK],
                        start=False, stop=True,
                    )
                    nc.scalar.copy(out=qkv[:, pt, 0:QK], in_=ps)
                return emit

            def v_unit(pt):
                def emit():
                    ps = psVp.tile([128, GC], F32)
                    for ct in range(CT):
                        nc.tensor.matmul(
                            out=ps,
                            lhsT=xgs[n][:, ct, pt * 128 : (pt + 1) * 128],
                            rhs=wqkv_sb[:, ct, QK:QKV],
                            start=(ct == 0), stop=False,
                        )
                    nc.tensor.matmul(
                        out=ps, lhsT=ones1, rhs=bqkv_sb[:, QK:QKV],
                        start=False, stop=True,
                    )
                    nc.vector.tensor_copy(out=qkv[:, pt, QK:QKV], in_=ps)
                return emit

            h0 = n * 8
            def w_row():
                nc.gpsimd.dma_start(
                    out=qkR_w[:, n * 8 : (n + 1) * 8, :],
                    in_=qkv[:, :, 0:QK],
                )
            def w_col():
                nc.gpsimd.dma_start(
                    out=qkvC_w[:, h0 : h0 + 8, :], in_=qkv,
                )

            for pt in range(8):
                units.append(qk_unit(pt))
                if pt == 7:
                    units.append(w_row)
                units.append(v_unit(pt))
            units.append(w_col)
            return units

        def emit_transposes(m):
            qkt = qktpool.tile([128, CT, 1024], BF16, name="qkt")
            qkts[m] = qkt
            for cht in range(CT):
                nc.sync.dma_start_transpose(
                    out=qkt[:, cht, :],
                    in_=qkR[m * 1024 : (m + 1) * 1024,
                            cht * 128 : (cht + 1) * 128],
                )

        def row_drain(m):
            h0 = m * 8
            def drain(p, psO_t, rz_t):
                nc.vector.tensor_tensor(
                    out=O_sb[p][:, :].rearrange("q (w h) -> q h w", h=H)[
                        :, h0 : h0 + 8, :
                    ],
                    in0=psO_t[:, :].rearrange("q (h w) -> q h w", h=8),
                    in1=rz_t[:, :].rearrange("q (h w) -> q h w", h=8),
                    op=MULT,
                )
            return drain

        for n in range(NCH + 2):
            if 0 <= n - 1 < NCH:
                emit_transposes(n - 1)
            p1 = make_p1_units(n) if n < NCH else []
            if n >= 2:
                m = n - 2
                at = make_attn_units(
                    qkts[m], qkvs[m], QK, pchpool, rzpool, psSp, psZp, psOp,
                    row_drain(m), s_halves=False,
                )
            else:
                at = []
            if p1:
                _interleave(p1, at)
            else:
                _interleave(at, [])

    # ---------------- col block: col attention + p3, lag-1 pipeline ----------
    out_r = out.ap().rearrange("(t p) s -> p t s", p=128)
    with (
        tc.tile_pool(name="c_qkt", bufs=3) as qktcpool,
        tc.tile_pool(name="c_v", bufs=3) as vlpool,
        tc.tile_pool(name="c_pch", bufs=4) as pchpool,
        tc.tile_pool(name="c_rz", bufs=3) as rzpool,
        tc.tile_pool(name="c_oc", bufs=4) as ocpool,
        tc.tile_pool(name="c_out", bufs=4) as outpool,
        tc.tile_pool(name="c_pss", bufs=2, space="PSUM") as psSp,
        tc.tile_pool(name="c_psz", bufs=2, space="PSUM") as psZp,
        tc.tile_pool(name="c_pso", bufs=1, space="PSUM") as psOp,
        tc.tile_pool(name="c_psf", bufs=1, space="PSUM") as psFp,
    ):
        loads = {}

        def load_chunk(j):
            vL = vlpool.tile([128, 8, GC], BF16, name="vL")
            nc.gpsimd.dma_start(
                out=vL,
                in_=qkvC[j * 1024 : (j + 1) * 1024, QK:QKV].rearrange(
                    "(t p) d -> p t d", p=128
                ),
            )
            qkt = qktcpool.tile([128, CT, 1024], BF16, name="qktc")
            for cht in range(CT):
                nc.sync.dma_start_transpose(
                    out=qkt[:, cht, :],
                    in_=qkvC[j * 1024 : (j + 1) * 1024,
                             cht * 128 : (cht + 1) * 128],
                )
            loads[j] = (qkt, vL)

        load_chunk(0)
        load_chunk(1)

        def col_drain(j):
            def drain(p, psO_t, rz_t):
                oc = ocpool.tile([128, 1024], BF16, name="oc")
                nc.vector.tensor_tensor(out=oc, in0=psO_t, in1=rz_t, op=MULT)
                dst = O_sb[p][:, j * 1024 : (j + 1) * 1024]
                nc.gpsimd.tensor_tensor(out=dst, in0=dst, in1=oc, op=ADD)
            return drain

        def make_p3_units(m):
            units = []
            outsbs = {}

            def f_unit(pg, otp):
                def emit():
                    if otp == 0:
                        outsbs[pg] = outpool.tile(
                            [128, CT, 512], BF16, name="outsb"
                        )
                    psf = psFp.tile([128, 1024], F32)
                    off = m * 1024 + pg * 512
                    for oti in range(2):
                        ot = 2 * otp + oti
                        for ic in range(2):
                            nc.tensor.matmul(
                                out=psf[:, oti * 512 : (oti + 1) * 512],
                                lhsT=wout_sb[:, ic, ot * 128 : (ot + 1) * 128],
                                rhs=O_sb[ic][:, off : off + 512],
                                start=(ic == 0), stop=(ic == 1),
                            )
                    # bias (bout) is added on the host
                    if otp == 0:
                        nc.scalar.copy(out=outsbs[pg][:, 0:2, :], in_=psf)
                    else:
                        nc.vector.tensor_copy(
                            out=outsbs[pg][:, 2:4, :], in_=psf
                        )
                        nc.gpsimd.dma_start(
                            out=out_r[:, :, off : off + 512], in_=outsbs[pg]
                        )
                return emit

            for pg in range(2):
                for otp in range(2):
                    units.append(f_unit(pg, otp))
            return units

        for j in range(NCH + 2):
            units = []
            if j < NCH:
                def prefetch(jj=j):
                    if jj + 2 < NCH:
                        load_chunk(jj + 2)
                qkt, vL = loads[j]
                at = make_attn_units(
                    qkt, vL, 0, pchpool, rzpool, psSp, psZp, psOp,
                    col_drain(j), s_halves=True,
                )
                # prefetch after S0 so Pool's O_sb adds aren't queued
                # behind next chunk's vL load
                units = at[:1] + [prefetch] + at[1:]
            # p3 runs 2 chunks behind; O_sb dep tracking is whole-tile, so
            # ALL p3 units must be emitted before this chunk's first PV
            # drain-add or their O_sb reads wait on it (PE head-of-line)
            p3 = make_p3_units(j - 2) if j >= 2 else []
            if units:
                # units: [S0, prefetch, S1, Z0, S2, P0, S3, Z1, P1]
                for u in units[:3]:
                    u()
                for u in p3:
                    u()
                for u in units[3:]:
                    u()
            else:
                _interleave(p3, [])


def get_nc():
    global _CACHED_NC
    if _CACHED_NC is None:
        _CACHED_NC = build_nc()
    return _CACHED_NC


def make_in_maps(x, Wqkv, bqkv, Wout, bout):
    """Per-core input dicts: core c = (b, g) with b = c // 2, g = c % 2."""
    bf16 = ml_dtypes.bfloat16
    in_maps = []
    for c in range(8):
        b, g = c // 2, c % 2
        sel = slice(256 * g, 256 * (g + 1))
        wq = Wqkv[sel, :]
        wk = Wqkv[512 + 256 * g : 512 + 256 * (g + 1), :]
        wv = Wqkv[1024 + 256 * g : 1024 + 256 * (g + 1), :]
        bq = bqkv[sel]
        bk = bqkv[512 + 256 * g : 512 + 256 * (g + 1)]
        bvv = bqkv[1024 + 256 * g : 1024 + 256 * (g + 1)]
        w_all = np.concatenate([wq, wk, wv], axis=0)      # [768, 512]
        b_all = np.concatenate([bq, bk, bvv])             # [768]
        in_maps.append(
            {
                "x": np.ascontiguousarray(x[b].reshape(C, S)).astype(bf16),
                "wqkvT": np.ascontiguousarray(w_all.T).astype(bf16),
                "bqkv": b_all.reshape(1, QKV).astype(bf16),
                "woutT": np.ascontiguousarray(Wout[:, sel].T).astype(bf16),
            }
        )
    return in_maps


def assemble_output(results, B, bout):
    """results: list of 8 per-core dicts with 'out' [C, S] bf16 in col-major
    pixel order (no bias). Returns [B, C, H, W] float32 with bout added."""
    out = np.empty((B, C, H, W), dtype=np.float32)
    for b in range(B):
        acc = (
            results[2 * b]["out"].astype(np.float32)
            + results[2 * b + 1]["out"].astype(np.float32)
        )
        out[b] = acc.reshape(C, W, H).swapaxes(1, 2)
    out += np.asarray(bout, np.float32)[None, :, None, None]
    return out


def kernel(x, Wqkv, bqkv, Wout, bout):
    x = np.asarray(x, dtype=np.float32)
    Wqkv = np.asarray(Wqkv, dtype=np.float32)
    bqkv = np.asarray(bqkv, dtype=np.float32)
    Wout = np.asarray(Wout, dtype=np.float32)
    bout = np.asarray(bout, dtype=np.float32)

    nc = get_nc()
    in_maps = make_in_maps(x, Wqkv, bqkv, Wout, bout)
    res = run_bass_kernel_spmd(nc, in_maps, core_ids=list(range(8)))
    return assemble_output(res.results, x.shape[0], bout)

